# revision 1
# baseline (speedup 1.0000x reference)
"""MoE (top-2 of 8 experts) Trainium2 kernel.

Strategy: expert-parallel across 8 NeuronCores. The router (8192x1024 @
1024x8 + top-k) is tiny, so it runs on host in float64 (verified to
reproduce the fp32 reference ranking). Each core gets the tokens routed
to its expert (capacity 2304 >= observed max 2203) and runs the dense
2-layer FFN with float32r matmuls (full PE rate at N>=256, ~1.5e-4
matmul error) and exact-erf Gelu on ScalarE; the host scatter-adds the
two expert contributions per token.

Device layout: stage 1 computes h.T = gelu(W1.T @ x.T + b1) with W1
blocks stationary; stage 2 uses h.T 128x128 blocks as the stationary
operand streaming two 512-wide W2 chunks per load (halving weight-load
count) and produces y directly in [token, E] layout. The 1/k scale is
folded into W2 on host (exact for k=2); b2/k is added with a DVE
tensor_add from a partition-replicated tile.
"""

import sys

sys.path.insert(0, "/opt/trn_rl_repo")

import math

import numpy as np

_B, _SEQ, _E, _H, _NE = 4, 2048, 1024, 1536, 8
_T = _B * _SEQ
_CAP = 2304  # per-expert token capacity (multiple of 128; >= max count 2203)
_NCORES = 8
_P = 128

_nc_cache: dict = {}
_LOOP_BODY_REPS = [1]  # timing-only knob for the For_i variant


def _build_nc(
    inv_k: float,
    repeat: int = 1,
    loop_n: int = 0,
    ps2_bufs: int = 2,
    cb_pair: bool = False,
    rowsplit: bool = False,
    s1_ilv: bool = False,
    s2_deilv: bool = False,
    h_split: bool = True,
    xt_split: bool = True,
):
    """repeat>1 repeats the compute (timing); repeat=0 builds an I/O-identical
    near-no-op NEFF used as the timing baseline; loop_n>0 wraps the pass in a
    device-side For_i loop (timing only)."""
    from contextlib import ExitStack

    import concourse.tile as tile
    from concourse import bacc, mybir

    f32 = mybir.dt.float32
    f32r = mybir.dt.float32r
    KO1 = _E // _P   # 8  k-tiles for layer-1 contraction
    HT = _H // _P    # 12 h-tiles (layer-1 out / layer-2 contraction)
    EC = _E // 512   # 2  512-wide E chunks in stage 2
    colt = [(0, 512), (512, 512), (1024, 512), (1536, 512), (2048, 256)]

    nc = bacc.Bacc("TRN2", target_bir_lowering=False, debug=False)
    xt_d = nc.dram_tensor("xt", [_E, _CAP], f32r, kind="ExternalInput").ap()
    w1_d = nc.dram_tensor("w1", [_E, _H], f32r, kind="ExternalInput").ap()
    # w2k = W2 / k (host-scaled; exact for k a power of two)
    w2_d = nc.dram_tensor("w2k", [_H, _E], f32r, kind="ExternalInput").ap()
    b1_d = nc.dram_tensor("b1p", [_P, HT], f32, kind="ExternalInput").ap()
    # b2r = b2 / k replicated across partitions
    b2_d = nc.dram_tensor("b2r", [_P, _E], f32, kind="ExternalInput").ap()
    # y in natural [token, E] layout
    y_d = nc.dram_tensor("y", [_CAP, _E], f32, kind="ExternalOutput").ap()

    with tile.TileContext(nc) as tc:
        with ExitStack() as ctx:
            wpool = ctx.enter_context(tc.tile_pool(name="w", bufs=1))
            cpool = ctx.enter_context(tc.tile_pool(name="c", bufs=1))
            xpool = ctx.enter_context(tc.tile_pool(name="x", bufs=2))
            hpool = ctx.enter_context(tc.tile_pool(name="h", bufs=2))
            ypool = ctx.enter_context(tc.tile_pool(name="y", bufs=4))
            ps1 = ctx.enter_context(
                tc.tile_pool(name="ps1", bufs=8 - ps2_bufs, space="PSUM")
            )
            ps2 = ctx.enter_context(
                tc.tile_pool(name="ps2", bufs=ps2_bufs, space="PSUM")
            )

            if repeat == 0 and not loop_n:
                t = cpool.tile([_P, 4], f32, tag="nop")
                nc.gpsimd.dma_start(t[:], b1_d[:, 0:4])
                o = cpool.tile([_P, 4], f32, tag="nop_o")
                nc.vector.tensor_copy(o[:], t[:])
                nc.gpsimd.dma_start(y_d[0:_P, 0:4], o[:])
            else:
                w1_sb = [
                    wpool.tile([_P, _H], f32r, tag=f"w1_{ko}", name=f"w1_{ko}")
                    for ko in range(KO1)
                ]
                w2_sb = [
                    wpool.tile([_P, _E], f32r, tag=f"w2_{hi}", name=f"w2_{hi}")
                    for hi in range(HT)
                ]
                b1_sb = cpool.tile([_P, HT], f32, tag="b1")
                b2_sb = cpool.tile([_P, _E], f32, tag="b2")

                def dma_xt(c0, nt, tag2, interleave_with_w1=False):
                    if xt_split:
                        tiles = []
                        for ko in range(KO1):
                            if interleave_with_w1:
                                nc.gpsimd.dma_start(
                                    w1_sb[ko][:], w1_d[ko * _P : (ko + 1) * _P, :]
                                )
                            tt = xpool.tile([_P, nt], f32r, tag=f"xt{ko}",
                                            name=f"xt_{tag2}_{ko}")
                            nc.gpsimd.dma_start(
                                tt[:], xt_d[ko * _P : (ko + 1) * _P, c0 : c0 + nt]
                            )
                            tiles.append(tt)
                        return lambda ko: tiles[ko][:]
                    t = xpool.tile([_P, KO1 * nt], f32r, tag="xt", name=f"xt_{tag2}")
                    for ko in range(KO1):
                        if interleave_with_w1:
                            nc.gpsimd.dma_start(
                                w1_sb[ko][:], w1_d[ko * _P : (ko + 1) * _P, :]
                            )
                        nc.gpsimd.dma_start(
                            t[:, ko * nt : (ko + 1) * nt],
                            xt_d[ko * _P : (ko + 1) * _P, c0 : c0 + nt],
                        )
                    return lambda ko: t[:, ko * nt : (ko + 1) * nt]

                # DMA issue order: w1/xt0 interleaved, b1, w2, b2 — what the
                # PE needs first arrives first.
                xt0 = dma_xt(colt[0][0], colt[0][1], "t0", interleave_with_w1=True)
                nc.gpsimd.dma_start(b1_sb[:], b1_d[:, :])
                for hi in range(HT):
                    nc.gpsimd.dma_start(
                        w2_sb[hi][:], w2_d[hi * _P : (hi + 1) * _P, :]
                    )
                nc.gpsimd.dma_start(b2_sb[:], b2_d[:, :])

                def alloc_h(nt, tag2):
                    if h_split:
                        tiles = [
                            hpool.tile([_P, nt], f32r, tag=f"h{hi}",
                                       name=f"h_{tag2}_{hi}")
                            for hi in range(HT)
                        ]
                        return (lambda hi: tiles[hi][:],
                                lambda hk, cb: tiles[hk][:, cb * _P:(cb + 1) * _P])
                    t = hpool.tile([_P, HT * nt], f32r, tag="h", name=f"h_{tag2}")
                    return (lambda hi: t[:, hi * nt:(hi + 1) * nt],
                            lambda hk, cb: t[:, hk * nt + cb * _P: hk * nt + (cb + 1) * _P])

                def stage1_koouter(xt_sb, nt):
                    # ko-outer half-passes: PE starts as soon as w1 block 0 lands
                    h_w, h_r = alloc_h(nt, "t0")
                    for half in range(2):
                        accs = [
                            ps1.tile([_P, nt], f32, tag="ps1", name=f"ps_h{half}_{i}")
                            for i in range(6)
                        ]
                        for ko in range(KO1):
                            for i in range(6):
                                hi = half * 6 + i
                                nc.tensor.matmul(
                                    accs[i][:],
                                    w1_sb[ko][:, hi * _P : (hi + 1) * _P],
                                    xt_sb(ko),
                                    start=(ko == 0),
                                    stop=(ko == KO1 - 1),
                                )
                        for i in range(6):
                            hi = half * 6 + i
                            nc.scalar.activation(
                                h_w(hi),
                                accs[i][:],
                                mybir.ActivationFunctionType.Gelu,
                                bias=b1_sb[:, hi : hi + 1],
                                scale=1.0,
                            )
                    return h_w, h_r

                def stage1(xt_sb, nt, tag2):
                    h_w, h_r = alloc_h(nt, tag2)
                    if s1_ilv:
                        # interleave pairs of accumulation chains across 2 banks
                        for hp in range(0, HT, 2):
                            pair = [hp, hp + 1]
                            accs = {
                                hi: ps1.tile(
                                    [_P, nt], f32, tag="ps1", name=f"p1_{tag2}_{hi}"
                                )
                                for hi in pair
                            }
                            for ko in range(KO1):
                                rhs = xt_sb(ko)
                                for hi in pair:
                                    nc.tensor.matmul(
                                        accs[hi][:],
                                        w1_sb[ko][:, hi * _P : (hi + 1) * _P],
                                        rhs,
                                        start=(ko == 0),
                                        stop=(ko == KO1 - 1),
                                    )
                            for hi in pair:
                                nc.scalar.activation(
                                    h_w(hi),
                                    accs[hi][:],
                                    mybir.ActivationFunctionType.Gelu,
                                    bias=b1_sb[:, hi : hi + 1],
                                    scale=1.0,
                                )
                        return h_w, h_r
                    for hi in range(HT):
                        acc = ps1.tile([_P, nt], f32, tag="ps1", name=f"p1_{tag2}_{hi}")
                        for ko in range(KO1):
                            lhs = w1_sb[ko][:, hi * _P : (hi + 1) * _P]
                            rhs = xt_sb(ko)
                            if rowsplit:
                                nc.tensor.matmul(
                                    acc[:], lhs[0:64, :], rhs[0:64, :],
                                    start=(ko == 0), stop=False,
                                )
                                nc.tensor.matmul(
                                    acc[:], lhs[64:128, :], rhs[64:128, :],
                                    start=False, stop=(ko == KO1 - 1),
                                )
                            else:
                                nc.tensor.matmul(
                                    acc[:], lhs, rhs,
                                    start=(ko == 0),
                                    stop=(ko == KO1 - 1),
                                )
                        nc.scalar.activation(
                            h_w(hi),
                            acc[:],
                            mybir.ActivationFunctionType.Gelu,
                            bias=b1_sb[:, hi : hi + 1],
                            scale=1.0,
                        )
                    return h_w, h_r

                def _emit_y(cbs, accs, c0, nt, tag2):
                    for cb in cbs:
                        for ec in range(EC):
                            y_sb = ypool.tile(
                                [_P, 512], f32, tag="y", name=f"y_{tag2}_{cb}_{ec}"
                            )
                            nc.vector.tensor_add(
                                y_sb[:],
                                accs[(cb, ec)][:],
                                b2_sb[:, ec * 512 : (ec + 1) * 512],
                            )
                            nc.gpsimd.dma_start(
                                y_d[
                                    c0 + cb * _P : c0 + (cb + 1) * _P,
                                    ec * 512 : (ec + 1) * 512,
                                ],
                                y_sb[:],
                            )

                def _stage2_cbs(h_r, c0, nt, tag2, cbs):
                    accs = {
                        (cb, ec): ps2.tile(
                            [_P, 512], f32, tag="ps2", name=f"p2_{tag2}_{cb}_{ec}"
                        )
                        for cb in cbs
                        for ec in range(EC)
                    }
                    if s2_deilv:
                        # one 12-deep chain per bank, no bank alternation
                        for cb in cbs:
                            for ec in range(EC):
                                for hk in range(HT):
                                    lhs = h_r(hk, cb)
                                    nc.tensor.matmul(
                                        accs[(cb, ec)][:],
                                        lhs,
                                        w2_sb[hk][:, ec * 512 : (ec + 1) * 512],
                                        start=(hk == 0),
                                        stop=(hk == HT - 1),
                                    )
                        _emit_y(cbs, accs, c0, nt, tag2)
                        return
                    for hk in range(HT):
                        for cb in cbs:
                            lhs = h_r(hk, cb)
                            for ec in range(EC):
                                rhs = w2_sb[hk][:, ec * 512 : (ec + 1) * 512]
                                if rowsplit:
                                    nc.tensor.matmul(
                                        accs[(cb, ec)][:], lhs[0:64, :], rhs[0:64, :],
                                        start=(hk == 0), stop=False,
                                    )
                                    nc.tensor.matmul(
                                        accs[(cb, ec)][:],
                                        lhs[64:128, :], rhs[64:128, :],
                                        start=False, stop=(hk == HT - 1),
                                    )
                                else:
                                    nc.tensor.matmul(
                                        accs[(cb, ec)][:], lhs, rhs,
                                        start=(hk == 0),
                                        stop=(hk == HT - 1),
                                    )
                    _emit_y(cbs, accs, c0, nt, tag2)

                def stage2(h_r, c0, nt, tag2):
                    # stationary = h.T 128x128 block (one load, 2 MMs of N=512)
                    ncb = nt // _P
                    if cb_pair:
                        for cb0 in range(0, ncb, 2):
                            _stage2_cbs(
                                h_r, c0, nt, tag2,
                                list(range(cb0, min(cb0 + 2, ncb))),
                            )
                    else:
                        for cb in range(ncb):
                            _stage2_cbs(h_r, c0, nt, tag2, [cb])

                if loop_n:
                    body_reps = int(_LOOP_BODY_REPS[0])
                    with tc.For_i(0, loop_n, 1) as _i:
                        for br in range(body_reps):
                            for c0, nt in colt:
                                xt_sb = dma_xt(c0, nt, f"L{br}_{c0}")
                                _, h_r = stage1(xt_sb, nt, f"L{br}_{c0}")
                                stage2(h_r, c0, nt, f"L{br}_{c0}")
                    repeat = 0  # body already emitted

                for r in range(repeat):
                    # emission: t0 s1, t1 s1, t0 s2, t1 s2 — covers w2's DMA
                    # arrival with tile-1 stage-1 PE work
                    if r == 0:
                        _, h0r = stage1_koouter(xt0, colt[0][1])
                    else:
                        _, h0r = stage1(
                            dma_xt(colt[0][0], colt[0][1], f"r{r}t0"),
                            colt[0][1],
                            f"r{r}t0",
                        )
                    xt1 = dma_xt(colt[1][0], colt[1][1], f"r{r}t1")
                    _, h1r = stage1(xt1, colt[1][1], f"r{r}t1")
                    stage2(h0r, colt[0][0], colt[0][1], f"r{r}t0")
                    stage2(h1r, colt[1][0], colt[1][1], f"r{r}t1")
                    for c0, nt in colt[2:]:
                        xt_sb = dma_xt(c0, nt, f"r{r}c{c0}")
                        _, h_r = stage1(xt_sb, nt, f"r{r}c{c0}")
                        stage2(h_r, c0, nt, f"r{r}c{c0}")
    nc.compile()
    return nc


def _make_runner(nc, n_cores):
    """Persistent-jit SPMD runner (modeled on bass2jax.run_bass_via_pjrt)."""
    import jax
    import numpy as _np
    from jax.sharding import Mesh, PartitionSpec
    from jax.experimental.shard_map import shard_map

    from concourse import mybir
    from concourse.bass2jax import (
        _bass_exec_p,
        install_neuronx_cc_hook,
        partition_id_tensor,
    )

    install_neuronx_cc_hook()

    partition_name = nc.partition_id_tensor.name if nc.partition_id_tensor else None
    in_names: list = []
    out_names: list = []
    out_avals: list = []
    zero_outs: list = []
    for alloc in nc.m.functions[0].allocations:
        if not isinstance(alloc, mybir.MemoryLocationSet):
            continue
        name = alloc.memorylocations[0].name
        if alloc.kind == "ExternalInput":
            if name != partition_name:
                in_names.append(name)
        elif alloc.kind == "ExternalOutput":
            shape = tuple(alloc.tensor_shape)
            dtype = mybir.dt.np(alloc.dtype)
            out_names.append(name)
            out_avals.append(jax.core.ShapedArray(shape, dtype))
            zero_outs.append(_np.zeros(shape, dtype))
    n_params = len(in_names)
    n_outs = len(out_avals)
    all_in_names = in_names + out_names
    if partition_name is not None:
        all_in_names = all_in_names + [partition_name]

    def _body(*args):
        operands = list(args)
        if partition_name is not None:
            operands.append(partition_id_tensor())
        outs = _bass_exec_p.bind(
            *operands,
            out_avals=tuple(out_avals),
            in_names=tuple(all_in_names),
            out_names=tuple(out_names),
            lowering_input_output_aliases=(),
            sim_require_finite=True,
            sim_require_nnan=True,
            nc=nc,
        )
        return tuple(outs)

    devices = jax.devices()[:n_cores]
    assert len(devices) == n_cores
    mesh = Mesh(_np.asarray(devices), ("core",))
    in_specs = (PartitionSpec("core"),) * (n_params + n_outs)
    out_specs = (PartitionSpec("core"),) * n_outs
    donate = tuple(range(n_params, n_params + n_outs))
    sharded = jax.jit(
        shard_map(
            _body, mesh=mesh, in_specs=in_specs, out_specs=out_specs, check_rep=False
        ),
        donate_argnums=donate,
        keep_unused=True,
    )

    def run(in_maps):
        concat_in = [
            _np.concatenate([_np.asarray(in_maps[c][nm]) for c in range(n_cores)], axis=0)
            for nm in in_names
        ]
        concat_zeros = [
            _np.zeros((n_cores * z.shape[0], *z.shape[1:]), z.dtype) for z in zero_outs
        ]
        out_arrs = sharded(*concat_in, *concat_zeros)
        out_arrs = [_np.asarray(o) for o in out_arrs]
        return [
            {
                nm: out_arrs[i].reshape(n_cores, *out_avals[i].shape)[c]
                for i, nm in enumerate(out_names)
            }
            for c in range(n_cores)
        ]

    return run


def _route(flat, Wr, br, k):
    logits = flat.astype(np.float64) @ Wr.astype(np.float64) + br.astype(np.float64)
    order = np.argsort(-logits, axis=1, kind="stable")
    return order[:, :k]


def _host_expert(xe, W1e, b1e, W2e, b2e):
    h = xe.astype(np.float64) @ W1e.astype(np.float64) + b1e.astype(np.float64)
    try:
        from scipy.special import erf
    except ImportError:
        erf = np.vectorize(math.erf)
    h = 0.5 * h * (1.0 + erf(h / math.sqrt(2.0)))
    return h @ W2e.astype(np.float64) + b2e.astype(np.float64)


def _have_axon_devices():
    try:
        import jax

        return (
            sum(d.platform in ("axon", "neuron") for d in jax.devices()) >= _NCORES
        )
    except Exception:
        return False


def _prepare(inputs):
    x = np.asarray(inputs["x"], np.float32)
    Wr = np.asarray(inputs["Wr"], np.float32)
    br = np.asarray(inputs["br"], np.float32)
    W1 = np.asarray(inputs["W1"], np.float32)
    b1 = np.asarray(inputs["b1"], np.float32)
    W2 = np.asarray(inputs["W2"], np.float32)
    b2 = np.asarray(inputs["b2"], np.float32)
    k = int(np.asarray(inputs["k"]))
    assert x.shape == (_B, _SEQ, _E), x.shape

    flat = x.reshape(_T, _E)
    topk = _route(flat, Wr, br, k)
    flatT = np.ascontiguousarray(flat.T)

    in_maps = []
    idx_list = []
    overflow = []
    for e in range(_NE):
        idx = np.nonzero((topk == e).any(axis=1))[0]
        if len(idx) > _CAP:
            overflow.append((e, idx[_CAP:]))
            idx = idx[:_CAP]
        idx_list.append(idx)
        xt = np.zeros((_E, _CAP), np.float32)
        xt[:, : len(idx)] = flatT[:, idx]
        in_maps.append(
            {
                "xt": xt,
                "w1": np.ascontiguousarray(W1[e]),
                "w2k": np.ascontiguousarray(W2[e] / k),
                "b1p": np.ascontiguousarray(b1[e].reshape(_H // _P, _P).T),
                "b2r": np.broadcast_to(b2[e] / k, (_P, _E)).copy(),
            }
        )
    return flat, k, in_maps, idx_list, overflow, (W1, b1, W2, b2)


def kernel(**inputs) -> np.ndarray:
    flat, k, in_maps, idx_list, overflow, wb = _prepare(inputs)
    if not _have_axon_devices():
        # no trn2 cores visible — compute on host so we still return the
        # right answer
        W1, b1, W2, b2 = wb
        out = np.zeros((_T, _E), np.float64)
        for e in range(_NE):
            idx = idx_list[e]
            out[idx] += _host_expert(flat[idx], W1[e], b1[e], W2[e], b2[e]) / k
        for e, idx in overflow:
            out[idx] += _host_expert(flat[idx], W1[e], b1[e], W2[e], b2[e]) / k
        return out.astype(np.float32).reshape(_B, _SEQ, _E)
    if overflow:
        # recompute overflow rows fully on host (exact erf gelu)
        W1, b1, W2, b2 = wb
        extra = [(e, idx, _host_expert(flat[idx], W1[e], b1[e], W2[e], b2[e]) / k)
                 for e, idx in overflow]
    else:
        extra = []

    key = (float(1.0 / k),)
    if key not in _nc_cache:
        nc = _build_nc(1.0 / k)
        _nc_cache[key] = _make_runner(nc, _NCORES)
    run = _nc_cache[key]
    results = run(in_maps)

    out = np.zeros((_T, _E), np.float32)
    for e in range(_NE):
        y = results[e]["y"]
        n = len(idx_list[e])
        out[idx_list[e]] += y[:n]
    for e, idx, yv in extra:
        out[idx] += yv.astype(np.float32)
    return out.reshape(_B, _SEQ, _E)



# revision 4
# speedup vs baseline: 1.3394x; 1.3394x over previous
"""MoE (top-2 of 8 experts) Trainium2 kernel.

Strategy: expert-parallel across 8 NeuronCores. The router (8192x1024 @
1024x8 + top-k) is tiny, so it runs on host in float64 (verified to
reproduce the fp32 reference ranking). Each core gets the tokens routed
to its expert (capacity 2304 >= observed max 2203) and runs the dense
2-layer FFN with float32r matmuls (full PE rate at N>=256, ~1.5e-4
matmul error) and exact-erf Gelu on ScalarE; the host scatter-adds the
two expert contributions per token.

Device layout: stage 1 computes h.T = gelu(W1.T @ x.T + b1) with W1
blocks stationary; stage 2 uses h.T 128x128 blocks as the stationary
operand streaming two 512-wide W2 chunks per load (halving weight-load
count) and produces y directly in [token, E] layout. The 1/k scale is
folded into W2 on host (exact for k=2); b2/k is added with a DVE
tensor_add from a partition-replicated tile.
"""

import sys

sys.path.insert(0, "/opt/trn_rl_repo")

import math

import numpy as np

_B, _SEQ, _E, _H, _NE = 4, 2048, 1024, 1536, 8
_T = _B * _SEQ
_CAP = 2304  # per-expert token capacity (multiple of 128; >= max count 2203)
_NCORES = 8
_P = 128

_nc_cache: dict = {}
_LOOP_BODY_REPS = [1]  # timing-only knob for the For_i variant


def _build_nc(
    inv_k: float,
    repeat: int = 1,
    loop_n: int = 0,
    ps2_bufs: int = 2,
    cb_pair: bool = False,
    rowsplit: bool = False,
    s1_ilv: bool = False,
    s2_deilv: bool = False,
    h_split: bool = True,
    xt_split: bool = True,
):
    """repeat>1 repeats the compute (timing); repeat=0 builds an I/O-identical
    near-no-op NEFF used as the timing baseline; loop_n>0 wraps the pass in a
    device-side For_i loop (timing only)."""
    from contextlib import ExitStack

    import concourse.tile as tile
    from concourse import bacc, mybir

    f32 = mybir.dt.float32
    f32r = mybir.dt.bfloat16  # matmul dtype (bf16: full PE rate + FWL + half DMA)
    KO1 = _E // _P   # 8  k-tiles for layer-1 contraction
    HT = _H // _P    # 12 h-tiles (layer-1 out / layer-2 contraction)
    EC = _E // 512   # 2  512-wide E chunks in stage 2
    colt = [(0, 512), (512, 512), (1024, 512), (1536, 512), (2048, 256)]

    nc = bacc.Bacc("TRN2", target_bir_lowering=False, debug=False)
    xt_d = nc.dram_tensor("xt", [_E, _CAP], f32r, kind="ExternalInput").ap()
    w1_d = nc.dram_tensor("w1", [_E, _H], f32r, kind="ExternalInput").ap()
    # w2k = W2 / k (host-scaled; exact for k a power of two)
    w2_d = nc.dram_tensor("w2k", [_H, _E], f32r, kind="ExternalInput").ap()
    b1_d = nc.dram_tensor("b1p", [_P, HT], f32, kind="ExternalInput").ap()
    # b2r = b2 / k replicated across partitions
    b2_d = nc.dram_tensor("b2r", [_P, _E], f32, kind="ExternalInput").ap()
    # y in natural [token, E] layout
    y_d = nc.dram_tensor("y", [_CAP, _E], f32, kind="ExternalOutput").ap()

    with tile.TileContext(nc) as tc:
        with ExitStack() as ctx:
            wpool = ctx.enter_context(tc.tile_pool(name="w", bufs=1))
            cpool = ctx.enter_context(tc.tile_pool(name="c", bufs=1))
            xpool = ctx.enter_context(tc.tile_pool(name="x", bufs=2))
            hpool = ctx.enter_context(tc.tile_pool(name="h", bufs=2))
            ypool = ctx.enter_context(tc.tile_pool(name="y", bufs=4))
            ps1 = ctx.enter_context(
                tc.tile_pool(name="ps1", bufs=8 - ps2_bufs, space="PSUM")
            )
            ps2 = ctx.enter_context(
                tc.tile_pool(name="ps2", bufs=ps2_bufs, space="PSUM")
            )

            if repeat == 0 and not loop_n:
                t = cpool.tile([_P, 4], f32, tag="nop")
                nc.gpsimd.dma_start(t[:], b1_d[:, 0:4])
                o = cpool.tile([_P, 4], f32, tag="nop_o")
                nc.vector.tensor_copy(o[:], t[:])
                nc.gpsimd.dma_start(y_d[0:_P, 0:4], o[:])
            else:
                w1_sb = [
                    wpool.tile([_P, _H], f32r, tag=f"w1_{ko}", name=f"w1_{ko}")
                    for ko in range(KO1)
                ]
                w2_sb = [
                    wpool.tile([_P, _E], f32r, tag=f"w2_{hi}", name=f"w2_{hi}")
                    for hi in range(HT)
                ]
                b1_sb = cpool.tile([_P, HT], f32, tag="b1")
                b2_sb = cpool.tile([_P, _E], f32, tag="b2")

                def dma_xt(c0, nt, tag2, interleave_with_w1=False):
                    if xt_split:
                        tiles = []
                        for ko in range(KO1):
                            if interleave_with_w1:
                                nc.gpsimd.dma_start(
                                    w1_sb[ko][:], w1_d[ko * _P : (ko + 1) * _P, :]
                                )
                            tt = xpool.tile([_P, nt], f32r, tag=f"xt{ko}",
                                            name=f"xt_{tag2}_{ko}")
                            nc.gpsimd.dma_start(
                                tt[:], xt_d[ko * _P : (ko + 1) * _P, c0 : c0 + nt]
                            )
                            tiles.append(tt)
                        return lambda ko: tiles[ko][:]
                    t = xpool.tile([_P, KO1 * nt], f32r, tag="xt", name=f"xt_{tag2}")
                    for ko in range(KO1):
                        if interleave_with_w1:
                            nc.gpsimd.dma_start(
                                w1_sb[ko][:], w1_d[ko * _P : (ko + 1) * _P, :]
                            )
                        nc.gpsimd.dma_start(
                            t[:, ko * nt : (ko + 1) * nt],
                            xt_d[ko * _P : (ko + 1) * _P, c0 : c0 + nt],
                        )
                    return lambda ko: t[:, ko * nt : (ko + 1) * nt]

                # DMA issue order: w1/xt0 interleaved, b1, w2, b2 — what the
                # PE needs first arrives first.
                xt0 = dma_xt(colt[0][0], colt[0][1], "t0", interleave_with_w1=True)
                nc.gpsimd.dma_start(b1_sb[:], b1_d[:, :])
                for hi in range(HT):
                    nc.gpsimd.dma_start(
                        w2_sb[hi][:], w2_d[hi * _P : (hi + 1) * _P, :]
                    )
                nc.gpsimd.dma_start(b2_sb[:], b2_d[:, :])

                def alloc_h(nt, tag2):
                    if h_split:
                        tiles = [
                            hpool.tile([_P, nt], f32r, tag=f"h{hi}",
                                       name=f"h_{tag2}_{hi}")
                            for hi in range(HT)
                        ]
                        return (lambda hi: tiles[hi][:],
                                lambda hk, cb: tiles[hk][:, cb * _P:(cb + 1) * _P])
                    t = hpool.tile([_P, HT * nt], f32r, tag="h", name=f"h_{tag2}")
                    return (lambda hi: t[:, hi * nt:(hi + 1) * nt],
                            lambda hk, cb: t[:, hk * nt + cb * _P: hk * nt + (cb + 1) * _P])

                def stage1_koouter(xt_sb, nt):
                    # ko-outer half-passes: PE starts as soon as w1 block 0 lands
                    h_w, h_r = alloc_h(nt, "t0")
                    for half in range(2):
                        accs = [
                            ps1.tile([_P, nt], f32, tag="ps1", name=f"ps_h{half}_{i}")
                            for i in range(6)
                        ]
                        for ko in range(KO1):
                            for i in range(6):
                                hi = half * 6 + i
                                nc.tensor.matmul(
                                    accs[i][:],
                                    w1_sb[ko][:, hi * _P : (hi + 1) * _P],
                                    xt_sb(ko),
                                    start=(ko == 0),
                                    stop=(ko == KO1 - 1),
                                )
                        for i in range(6):
                            hi = half * 6 + i
                            nc.scalar.activation(
                                h_w(hi),
                                accs[i][:],
                                mybir.ActivationFunctionType.Gelu,
                                bias=b1_sb[:, hi : hi + 1],
                                scale=1.0,
                            )
                    return h_w, h_r

                def stage1(xt_sb, nt, tag2):
                    h_w, h_r = alloc_h(nt, tag2)
                    if s1_ilv:
                        # interleave pairs of accumulation chains across 2 banks
                        for hp in range(0, HT, 2):
                            pair = [hp, hp + 1]
                            accs = {
                                hi: ps1.tile(
                                    [_P, nt], f32, tag="ps1", name=f"p1_{tag2}_{hi}"
                                )
                                for hi in pair
                            }
                            for ko in range(KO1):
                                rhs = xt_sb(ko)
                                for hi in pair:
                                    nc.tensor.matmul(
                                        accs[hi][:],
                                        w1_sb[ko][:, hi * _P : (hi + 1) * _P],
                                        rhs,
                                        start=(ko == 0),
                                        stop=(ko == KO1 - 1),
                                    )
                            for hi in pair:
                                nc.scalar.activation(
                                    h_w(hi),
                                    accs[hi][:],
                                    mybir.ActivationFunctionType.Gelu,
                                    bias=b1_sb[:, hi : hi + 1],
                                    scale=1.0,
                                )
                        return h_w, h_r
                    for hi in range(HT):
                        acc = ps1.tile([_P, nt], f32, tag="ps1", name=f"p1_{tag2}_{hi}")
                        for ko in range(KO1):
                            lhs = w1_sb[ko][:, hi * _P : (hi + 1) * _P]
                            rhs = xt_sb(ko)
                            if rowsplit:
                                nc.tensor.matmul(
                                    acc[:], lhs[0:64, :], rhs[0:64, :],
                                    start=(ko == 0), stop=False,
                                )
                                nc.tensor.matmul(
                                    acc[:], lhs[64:128, :], rhs[64:128, :],
                                    start=False, stop=(ko == KO1 - 1),
                                )
                            else:
                                nc.tensor.matmul(
                                    acc[:], lhs, rhs,
                                    start=(ko == 0),
                                    stop=(ko == KO1 - 1),
                                )
                        nc.scalar.activation(
                            h_w(hi),
                            acc[:],
                            mybir.ActivationFunctionType.Gelu,
                            bias=b1_sb[:, hi : hi + 1],
                            scale=1.0,
                        )
                    return h_w, h_r

                def _emit_y(cbs, accs, c0, nt, tag2):
                    for cb in cbs:
                        for ec in range(EC):
                            y_sb = ypool.tile(
                                [_P, 512], f32, tag="y", name=f"y_{tag2}_{cb}_{ec}"
                            )
                            nc.vector.tensor_add(
                                y_sb[:],
                                accs[(cb, ec)][:],
                                b2_sb[:, ec * 512 : (ec + 1) * 512],
                            )
                            nc.gpsimd.dma_start(
                                y_d[
                                    c0 + cb * _P : c0 + (cb + 1) * _P,
                                    ec * 512 : (ec + 1) * 512,
                                ],
                                y_sb[:],
                            )

                def _stage2_cbs(h_r, c0, nt, tag2, cbs):
                    accs = {
                        (cb, ec): ps2.tile(
                            [_P, 512], f32, tag="ps2", name=f"p2_{tag2}_{cb}_{ec}"
                        )
                        for cb in cbs
                        for ec in range(EC)
                    }
                    if s2_deilv:
                        # one 12-deep chain per bank, no bank alternation
                        for cb in cbs:
                            for ec in range(EC):
                                for hk in range(HT):
                                    lhs = h_r(hk, cb)
                                    nc.tensor.matmul(
                                        accs[(cb, ec)][:],
                                        lhs,
                                        w2_sb[hk][:, ec * 512 : (ec + 1) * 512],
                                        start=(hk == 0),
                                        stop=(hk == HT - 1),
                                    )
                        _emit_y(cbs, accs, c0, nt, tag2)
                        return
                    for hk in range(HT):
                        for cb in cbs:
                            lhs = h_r(hk, cb)
                            for ec in range(EC):
                                rhs = w2_sb[hk][:, ec * 512 : (ec + 1) * 512]
                                if rowsplit:
                                    nc.tensor.matmul(
                                        accs[(cb, ec)][:], lhs[0:64, :], rhs[0:64, :],
                                        start=(hk == 0), stop=False,
                                    )
                                    nc.tensor.matmul(
                                        accs[(cb, ec)][:],
                                        lhs[64:128, :], rhs[64:128, :],
                                        start=False, stop=(hk == HT - 1),
                                    )
                                else:
                                    nc.tensor.matmul(
                                        accs[(cb, ec)][:], lhs, rhs,
                                        start=(hk == 0),
                                        stop=(hk == HT - 1),
                                    )
                    _emit_y(cbs, accs, c0, nt, tag2)

                def stage2(h_r, c0, nt, tag2):
                    # stationary = h.T 128x128 block (one load, 2 MMs of N=512)
                    ncb = nt // _P
                    if cb_pair:
                        for cb0 in range(0, ncb, 2):
                            _stage2_cbs(
                                h_r, c0, nt, tag2,
                                list(range(cb0, min(cb0 + 2, ncb))),
                            )
                    else:
                        for cb in range(ncb):
                            _stage2_cbs(h_r, c0, nt, tag2, [cb])

                if loop_n:
                    body_reps = int(_LOOP_BODY_REPS[0])
                    with tc.For_i(0, loop_n, 1) as _i:
                        for br in range(body_reps):
                            for c0, nt in colt:
                                xt_sb = dma_xt(c0, nt, f"L{br}_{c0}")
                                _, h_r = stage1(xt_sb, nt, f"L{br}_{c0}")
                                stage2(h_r, c0, nt, f"L{br}_{c0}")
                    repeat = 0  # body already emitted

                for r in range(repeat):
                    # emission: t0 s1, t1 s1, t0 s2, t1 s2 — covers w2's DMA
                    # arrival with tile-1 stage-1 PE work
                    if r == 0:
                        _, h0r = stage1_koouter(xt0, colt[0][1])
                    else:
                        _, h0r = stage1(
                            dma_xt(colt[0][0], colt[0][1], f"r{r}t0"),
                            colt[0][1],
                            f"r{r}t0",
                        )
                    xt1 = dma_xt(colt[1][0], colt[1][1], f"r{r}t1")
                    _, h1r = stage1(xt1, colt[1][1], f"r{r}t1")
                    stage2(h0r, colt[0][0], colt[0][1], f"r{r}t0")
                    stage2(h1r, colt[1][0], colt[1][1], f"r{r}t1")
                    for c0, nt in colt[2:]:
                        xt_sb = dma_xt(c0, nt, f"r{r}c{c0}")
                        _, h_r = stage1(xt_sb, nt, f"r{r}c{c0}")
                        stage2(h_r, c0, nt, f"r{r}c{c0}")
    nc.compile()
    return nc


def _make_runner(nc, n_cores):
    """Persistent-jit SPMD runner (modeled on bass2jax.run_bass_via_pjrt)."""
    import jax
    import numpy as _np
    from jax.sharding import Mesh, PartitionSpec
    from jax.experimental.shard_map import shard_map

    from concourse import mybir
    from concourse.bass2jax import (
        _bass_exec_p,
        install_neuronx_cc_hook,
        partition_id_tensor,
    )

    install_neuronx_cc_hook()

    partition_name = nc.partition_id_tensor.name if nc.partition_id_tensor else None
    in_names: list = []
    out_names: list = []
    out_avals: list = []
    zero_outs: list = []
    for alloc in nc.m.functions[0].allocations:
        if not isinstance(alloc, mybir.MemoryLocationSet):
            continue
        name = alloc.memorylocations[0].name
        if alloc.kind == "ExternalInput":
            if name != partition_name:
                in_names.append(name)
        elif alloc.kind == "ExternalOutput":
            shape = tuple(alloc.tensor_shape)
            dtype = mybir.dt.np(alloc.dtype)
            out_names.append(name)
            out_avals.append(jax.core.ShapedArray(shape, dtype))
            zero_outs.append(_np.zeros(shape, dtype))
    n_params = len(in_names)
    n_outs = len(out_avals)
    all_in_names = in_names + out_names
    if partition_name is not None:
        all_in_names = all_in_names + [partition_name]

    def _body(*args):
        operands = list(args)
        if partition_name is not None:
            operands.append(partition_id_tensor())
        outs = _bass_exec_p.bind(
            *operands,
            out_avals=tuple(out_avals),
            in_names=tuple(all_in_names),
            out_names=tuple(out_names),
            lowering_input_output_aliases=(),
            sim_require_finite=True,
            sim_require_nnan=True,
            nc=nc,
        )
        return tuple(outs)

    devices = jax.devices()[:n_cores]
    assert len(devices) == n_cores
    mesh = Mesh(_np.asarray(devices), ("core",))
    in_specs = (PartitionSpec("core"),) * (n_params + n_outs)
    out_specs = (PartitionSpec("core"),) * n_outs
    donate = tuple(range(n_params, n_params + n_outs))
    sharded = jax.jit(
        shard_map(
            _body, mesh=mesh, in_specs=in_specs, out_specs=out_specs, check_rep=False
        ),
        donate_argnums=donate,
        keep_unused=True,
    )

    def run(in_maps):
        concat_in = [
            _np.concatenate([_np.asarray(in_maps[c][nm]) for c in range(n_cores)], axis=0)
            for nm in in_names
        ]
        concat_zeros = [
            _np.zeros((n_cores * z.shape[0], *z.shape[1:]), z.dtype) for z in zero_outs
        ]
        out_arrs = sharded(*concat_in, *concat_zeros)
        out_arrs = [_np.asarray(o) for o in out_arrs]
        return [
            {
                nm: out_arrs[i].reshape(n_cores, *out_avals[i].shape)[c]
                for i, nm in enumerate(out_names)
            }
            for c in range(n_cores)
        ]

    return run


def _route(flat, Wr, br, k):
    logits = flat.astype(np.float64) @ Wr.astype(np.float64) + br.astype(np.float64)
    order = np.argsort(-logits, axis=1, kind="stable")
    return order[:, :k]


def _host_expert(xe, W1e, b1e, W2e, b2e):
    h = xe.astype(np.float64) @ W1e.astype(np.float64) + b1e.astype(np.float64)
    try:
        from scipy.special import erf
    except ImportError:
        erf = np.vectorize(math.erf)
    h = 0.5 * h * (1.0 + erf(h / math.sqrt(2.0)))
    return h @ W2e.astype(np.float64) + b2e.astype(np.float64)


def _have_axon_devices():
    try:
        import jax

        return (
            sum(d.platform in ("axon", "neuron") for d in jax.devices()) >= _NCORES
        )
    except Exception:
        return False


def _prepare(inputs):
    import ml_dtypes

    bf16 = ml_dtypes.bfloat16
    x = np.asarray(inputs["x"], np.float32)
    Wr = np.asarray(inputs["Wr"], np.float32)
    br = np.asarray(inputs["br"], np.float32)
    W1 = np.asarray(inputs["W1"], np.float32)
    b1 = np.asarray(inputs["b1"], np.float32)
    W2 = np.asarray(inputs["W2"], np.float32)
    b2 = np.asarray(inputs["b2"], np.float32)
    k = int(np.asarray(inputs["k"]))
    assert x.shape == (_B, _SEQ, _E), x.shape

    flat = x.reshape(_T, _E)
    topk = _route(flat, Wr, br, k)
    flatT = np.ascontiguousarray(flat.T)

    in_maps = []
    idx_list = []
    overflow = []
    for e in range(_NE):
        idx = np.nonzero((topk == e).any(axis=1))[0]
        if len(idx) > _CAP:
            overflow.append((e, idx[_CAP:]))
            idx = idx[:_CAP]
        idx_list.append(idx)
        xt = np.zeros((_E, _CAP), bf16)
        xt[:, : len(idx)] = flatT[:, idx]
        in_maps.append(
            {
                "xt": xt,
                "w1": W1[e].astype(bf16),
                "w2k": (W2[e] / k).astype(bf16),
                "b1p": np.ascontiguousarray(b1[e].reshape(_H // _P, _P).T),
                "b2r": np.broadcast_to(b2[e] / k, (_P, _E)).copy(),
            }
        )
    return flat, k, in_maps, idx_list, overflow, (W1, b1, W2, b2)


def kernel(**inputs) -> np.ndarray:
    flat, k, in_maps, idx_list, overflow, wb = _prepare(inputs)
    if not _have_axon_devices():
        # no trn2 cores visible — compute on host so we still return the
        # right answer
        W1, b1, W2, b2 = wb
        out = np.zeros((_T, _E), np.float64)
        for e in range(_NE):
            idx = idx_list[e]
            out[idx] += _host_expert(flat[idx], W1[e], b1[e], W2[e], b2[e]) / k
        for e, idx in overflow:
            out[idx] += _host_expert(flat[idx], W1[e], b1[e], W2[e], b2[e]) / k
        return out.astype(np.float32).reshape(_B, _SEQ, _E)
    if overflow:
        # recompute overflow rows fully on host (exact erf gelu)
        W1, b1, W2, b2 = wb
        extra = [(e, idx, _host_expert(flat[idx], W1[e], b1[e], W2[e], b2[e]) / k)
                 for e, idx in overflow]
    else:
        extra = []

    key = (float(1.0 / k),)
    if key not in _nc_cache:
        nc = _build_nc(1.0 / k)
        _nc_cache[key] = _make_runner(nc, _NCORES)
    run = _nc_cache[key]
    results = run(in_maps)

    out = np.zeros((_T, _E), np.float32)
    for e in range(_NE):
        y = results[e]["y"]
        n = len(idx_list[e])
        out[idx_list[e]] += y[:n]
    for e, idx, yv in extra:
        out[idx] += yv.astype(np.float32)
    return out.reshape(_B, _SEQ, _E)



# revision 7
# speedup vs baseline: 1.5049x; 1.1236x over previous
"""MoE (top-2 of 8 experts) Trainium2 kernel.

Strategy: expert-parallel across 8 NeuronCores. The router (8192x1024 @
1024x8 + top-k) is tiny, so it runs on host in float64 (verified to
reproduce the fp32 reference ranking). Each core gets the tokens routed
to its expert (capacity 2304 >= observed max 2203) and runs the dense
2-layer FFN in bf16 (full PE rate, fast weight load, half the DMA bytes
of fp32; ~3e-3 rel err vs the 2e-2 gate) with exact-erf Gelu on ScalarE;
the host scatter-adds the two expert contributions per token.

Device layout: stage 1 computes h.T = gelu(W1.T @ x.T + b1) with W1
blocks stationary; stage 2 uses h.T 128x128 blocks as the stationary
operand streaming two 512-wide W2 chunks per load (halving weight-load
count) and produces y directly in [token, E] layout. The 1/k scale is
folded into W2 on host (exact for k=2); b2/k is added with a DVE
tensor_add from a partition-replicated tile.

DMA queues: token/weight loads go out on the sync engine's HWDGE ring;
y stores ride the otherwise-idle gpsimd (SWDGE) queue, so input
prefetch is never queued behind store triggers that wait on compute.
PSUM: 4 banks for stage-1 accumulation chains, 4 for stage-2 so
consecutive 128-token blocks alternate bank pairs and the PE never
waits on the DVE drain.
"""

import sys

sys.path.insert(0, "/opt/trn_rl_repo")

import math

import numpy as np

_B, _SEQ, _E, _H, _NE = 4, 2048, 1024, 1536, 8
_T = _B * _SEQ
_CAP = 2304  # per-expert token capacity (multiple of 128; >= max count 2203)
_NCORES = 8
_P = 128

_nc_cache: dict = {}
_LOOP_BODY_REPS = [1]  # timing-only knob for the For_i variant


def _build_nc(
    inv_k: float,
    repeat: int = 1,
    loop_n: int = 0,
    ps1_bufs: int = 4,
    ps2_bufs: int = 4,
    y_f32: bool = False,
):
    """repeat>1 repeats the compute (timing); repeat=0 builds an I/O-identical
    near-no-op NEFF used as the timing baseline; loop_n>0 wraps the pass in a
    device-side For_i loop (timing only)."""
    from contextlib import ExitStack

    import concourse.tile as tile
    from concourse import bacc, mybir

    f32 = mybir.dt.float32
    bf = mybir.dt.bfloat16
    ydt = f32 if y_f32 else bf
    KO1 = _E // _P   # 8  k-tiles for layer-1 contraction
    HT = _H // _P    # 12 h-tiles (layer-1 out / layer-2 contraction)
    EC = _E // 512   # 2  512-wide E chunks in stage 2
    colt = [(0, 512), (512, 512), (1024, 512), (1536, 512), (2048, 256)]

    nc = bacc.Bacc("TRN2", target_bir_lowering=False, debug=False)
    xt_d = nc.dram_tensor("xt", [_E, _CAP], bf, kind="ExternalInput").ap()
    w1_d = nc.dram_tensor("w1", [_E, _H], bf, kind="ExternalInput").ap()
    # w2k = W2 / k (host-scaled; exact for k a power of two)
    w2_d = nc.dram_tensor("w2k", [_H, _E], bf, kind="ExternalInput").ap()
    b1_d = nc.dram_tensor("b1p", [_P, HT], f32, kind="ExternalInput").ap()
    # b2r = b2 / k replicated across partitions
    b2_d = nc.dram_tensor("b2r", [_P, _E], f32, kind="ExternalInput").ap()
    # y in natural [token, E] layout
    y_d = nc.dram_tensor("y", [_CAP, _E], ydt, kind="ExternalOutput").ap()

    with tile.TileContext(nc) as tc:
        with ExitStack() as ctx:
            wpool = ctx.enter_context(tc.tile_pool(name="w", bufs=1))
            cpool = ctx.enter_context(tc.tile_pool(name="c", bufs=1))
            xpool = ctx.enter_context(tc.tile_pool(name="x", bufs=len(colt)))
            hpool = ctx.enter_context(tc.tile_pool(name="h", bufs=2))
            ypool = ctx.enter_context(tc.tile_pool(name="y", bufs=4))
            ps1 = ctx.enter_context(tc.tile_pool(name="ps1", bufs=ps1_bufs, space="PSUM"))
            ps2 = ctx.enter_context(tc.tile_pool(name="ps2", bufs=ps2_bufs, space="PSUM"))

            if repeat == 0 and not loop_n:
                t = cpool.tile([_P, 4], f32, tag="nop")
                nc.sync.dma_start(t[:], b1_d[:, 0:4])
                o = cpool.tile([_P, 4], f32, tag="nop_o")
                nc.vector.tensor_copy(o[:], t[:])
                nc.gpsimd.dma_start(y_d[0:_P, 0:4], o[:])
            else:
                w1_sb = [
                    wpool.tile([_P, _H], bf, tag=f"w1_{ko}", name=f"w1_{ko}")
                    for ko in range(KO1)
                ]
                w2_sb = [
                    wpool.tile([_P, _E], bf, tag=f"w2_{hi}", name=f"w2_{hi}")
                    for hi in range(HT)
                ]
                b1_sb = cpool.tile([_P, HT], f32, tag="b1")
                b2_sb = cpool.tile([_P, _E], f32, tag="b2")

                def dma_xt(c0, nt, tag2, interleave=None, ilv_first=False):
                    tiles = []
                    for ko in range(KO1):
                        if interleave is not None and ilv_first:
                            interleave(ko)
                        tt = xpool.tile([_P, nt], bf, tag=f"xt{ko}",
                                        name=f"xt_{tag2}_{ko}")
                        nc.sync.dma_start(
                            tt[:], xt_d[ko * _P : (ko + 1) * _P, c0 : c0 + nt]
                        )
                        if interleave is not None and not ilv_first:
                            interleave(ko)
                        tiles.append(tt)
                    return lambda ko: tiles[ko][:]

                def alloc_h(nt, tag2):
                    tiles = [
                        hpool.tile([_P, nt], bf, tag=f"h{hi}", name=f"h_{tag2}_{hi}")
                        for hi in range(HT)
                    ]
                    return (lambda hi: tiles[hi][:],
                            lambda hk, cb: tiles[hk][:, cb * _P : (cb + 1) * _P])

                def stage1_koouter(xt_sb, nt):
                    # ko-outer half-passes: PE starts as soon as w1 block 0 lands
                    h_w, h_r = alloc_h(nt, "t0")
                    for half in range(2):
                        accs = [
                            ps1.tile([_P, nt], f32, tag="ps1", name=f"ps_h{half}_{i}")
                            for i in range(6)
                        ]
                        for ko in range(KO1):
                            for i in range(6):
                                hi = half * 6 + i
                                nc.tensor.matmul(
                                    accs[i][:],
                                    w1_sb[ko][:, hi * _P : (hi + 1) * _P],
                                    xt_sb(ko),
                                    start=(ko == 0),
                                    stop=(ko == KO1 - 1),
                                )
                        for i in range(6):
                            hi = half * 6 + i
                            nc.scalar.activation(
                                h_w(hi),
                                accs[i][:],
                                mybir.ActivationFunctionType.Gelu,
                                bias=b1_sb[:, hi : hi + 1],
                                scale=1.0,
                            )
                    return h_w, h_r

                def stage1(xt_sb, nt, tag2):
                    h_w, h_r = alloc_h(nt, tag2)
                    for hi in range(HT):
                        acc = ps1.tile([_P, nt], f32, tag="ps1", name=f"p1_{tag2}_{hi}")
                        for ko in range(KO1):
                            nc.tensor.matmul(
                                acc[:],
                                w1_sb[ko][:, hi * _P : (hi + 1) * _P],
                                xt_sb(ko),
                                start=(ko == 0),
                                stop=(ko == KO1 - 1),
                            )
                        nc.scalar.activation(
                            h_w(hi),
                            acc[:],
                            mybir.ActivationFunctionType.Gelu,
                            bias=b1_sb[:, hi : hi + 1],
                            scale=1.0,
                        )
                    return h_w, h_r

                def stage2(h_r, c0, nt, tag2):
                    # stationary = h.T 128x128 block (one load, 2 MMs of N=512)
                    for cb in range(nt // _P):
                        accs = [
                            ps2.tile([_P, 512], f32, tag="ps2",
                                     name=f"p2_{tag2}_{cb}_{ec}")
                            for ec in range(EC)
                        ]
                        for hk in range(HT):
                            lhs = h_r(hk, cb)
                            for ec in range(EC):
                                nc.tensor.matmul(
                                    accs[ec][:],
                                    lhs,
                                    w2_sb[hk][:, ec * 512 : (ec + 1) * 512],
                                    start=(hk == 0),
                                    stop=(hk == HT - 1),
                                )
                        for ec in range(EC):
                            y_sb = ypool.tile([_P, 512], ydt, tag="y",
                                              name=f"y_{tag2}_{cb}_{ec}")
                            nc.vector.tensor_add(
                                y_sb[:],
                                accs[ec][:],
                                b2_sb[:, ec * 512 : (ec + 1) * 512],
                            )
                            nc.gpsimd.dma_start(
                                y_d[
                                    c0 + cb * _P : c0 + (cb + 1) * _P,
                                    ec * 512 : (ec + 1) * 512,
                                ],
                                y_sb[:],
                            )

                if loop_n:
                    # weights resident; body re-loads xt and writes y
                    for ko in range(KO1):
                        nc.sync.dma_start(w1_sb[ko][:], w1_d[ko * _P : (ko + 1) * _P, :])
                    nc.sync.dma_start(b1_sb[:], b1_d[:, :])
                    for hi in range(HT):
                        nc.sync.dma_start(w2_sb[hi][:], w2_d[hi * _P : (hi + 1) * _P, :])
                    nc.sync.dma_start(b2_sb[:], b2_d[:, :])
                    body_reps = int(_LOOP_BODY_REPS[0])
                    with tc.For_i(0, loop_n, 1) as _i:
                        for br in range(body_reps):
                            xts = [
                                dma_xt(c0, nt, f"L{br}_{c0}") for c0, nt in colt
                            ]
                            for t, (c0, nt) in enumerate(colt):
                                _, h_r = stage1(xts[t], nt, f"L{br}_{c0}")
                                stage2(h_r, c0, nt, f"L{br}_{c0}")
                else:
                    for r in range(repeat):
                        if r == 0:
                            # prelude issue order: w1/xt0 interleaved so PE can
                            # start immediately; then b1, then xt1..4 / w2
                            # interleaved, b2 — what the PE needs first
                            # arrives first.
                            xt0 = dma_xt(
                                colt[0][0], colt[0][1], "r0t0",
                                interleave=lambda ko: nc.sync.dma_start(
                                    w1_sb[ko][:], w1_d[ko * _P : (ko + 1) * _P, :]
                                ),
                                ilv_first=True,
                            )
                            nc.sync.dma_start(b1_sb[:], b1_d[:, :])
                            xts = [xt0]
                            w2i = iter(range(HT))

                            def ilv(ko):
                                hi = next(w2i, None)
                                if hi is not None:
                                    nc.sync.dma_start(
                                        w2_sb[hi][:],
                                        w2_d[hi * _P : (hi + 1) * _P, :],
                                    )

                            for c0, nt in colt[1:]:
                                xts.append(dma_xt(c0, nt, f"r0c{c0}", interleave=ilv))
                            nc.sync.dma_start(b2_sb[:], b2_d[:, :])
                            _, h0r = stage1_koouter(xts[0], colt[0][1])
                        else:
                            xts = [dma_xt(c0, nt, f"r{r}c{c0}") for c0, nt in colt]
                            _, h0r = stage1(xts[0], colt[0][1], f"r{r}t0")
                        # t0 s1, t1 s1, t0 s2, t1 s2 — covers w2's DMA arrival
                        # with tile-1 stage-1 PE work
                        _, h1r = stage1(xts[1], colt[1][1], f"r{r}t1")
                        stage2(h0r, colt[0][0], colt[0][1], f"r{r}t0")
                        stage2(h1r, colt[1][0], colt[1][1], f"r{r}t1")
                        for t, (c0, nt) in enumerate(colt[2:], start=2):
                            _, h_r = stage1(xts[t], nt, f"r{r}c{c0}")
                            stage2(h_r, c0, nt, f"r{r}c{c0}")
    nc.compile()
    return nc


def _make_runner(nc, n_cores):
    """Persistent-jit SPMD runner (modeled on bass2jax.run_bass_via_pjrt)."""
    import jax
    import numpy as _np
    from jax.sharding import Mesh, PartitionSpec
    from jax.experimental.shard_map import shard_map

    from concourse import mybir
    from concourse.bass2jax import (
        _bass_exec_p,
        install_neuronx_cc_hook,
        partition_id_tensor,
    )

    install_neuronx_cc_hook()

    partition_name = nc.partition_id_tensor.name if nc.partition_id_tensor else None
    in_names: list = []
    out_names: list = []
    out_avals: list = []
    zero_outs: list = []
    for alloc in nc.m.functions[0].allocations:
        if not isinstance(alloc, mybir.MemoryLocationSet):
            continue
        name = alloc.memorylocations[0].name
        if alloc.kind == "ExternalInput":
            if name != partition_name:
                in_names.append(name)
        elif alloc.kind == "ExternalOutput":
            shape = tuple(alloc.tensor_shape)
            dtype = mybir.dt.np(alloc.dtype)
            out_names.append(name)
            out_avals.append(jax.core.ShapedArray(shape, dtype))
            zero_outs.append(_np.zeros(shape, dtype))
    n_params = len(in_names)
    n_outs = len(out_avals)
    all_in_names = in_names + out_names
    if partition_name is not None:
        all_in_names = all_in_names + [partition_name]

    def _body(*args):
        operands = list(args)
        if partition_name is not None:
            operands.append(partition_id_tensor())
        outs = _bass_exec_p.bind(
            *operands,
            out_avals=tuple(out_avals),
            in_names=tuple(all_in_names),
            out_names=tuple(out_names),
            lowering_input_output_aliases=(),
            sim_require_finite=True,
            sim_require_nnan=True,
            nc=nc,
        )
        return tuple(outs)

    devices = jax.devices()[:n_cores]
    assert len(devices) == n_cores
    mesh = Mesh(_np.asarray(devices), ("core",))
    in_specs = (PartitionSpec("core"),) * (n_params + n_outs)
    out_specs = (PartitionSpec("core"),) * n_outs
    donate = tuple(range(n_params, n_params + n_outs))
    sharded = jax.jit(
        shard_map(
            _body, mesh=mesh, in_specs=in_specs, out_specs=out_specs, check_rep=False
        ),
        donate_argnums=donate,
        keep_unused=True,
    )

    def run(in_maps):
        concat_in = [
            _np.concatenate([_np.asarray(in_maps[c][nm]) for c in range(n_cores)], axis=0)
            for nm in in_names
        ]
        concat_zeros = [
            _np.zeros((n_cores * z.shape[0], *z.shape[1:]), z.dtype) for z in zero_outs
        ]
        out_arrs = sharded(*concat_in, *concat_zeros)
        out_arrs = [_np.asarray(o) for o in out_arrs]
        return [
            {
                nm: out_arrs[i].reshape(n_cores, *out_avals[i].shape)[c]
                for i, nm in enumerate(out_names)
            }
            for c in range(n_cores)
        ]

    return run


def _route(flat, Wr, br, k):
    logits = flat.astype(np.float64) @ Wr.astype(np.float64) + br.astype(np.float64)
    order = np.argsort(-logits, axis=1, kind="stable")
    return order[:, :k]


def _host_expert(xe, W1e, b1e, W2e, b2e):
    h = xe.astype(np.float64) @ W1e.astype(np.float64) + b1e.astype(np.float64)
    try:
        from scipy.special import erf
    except ImportError:
        erf = np.vectorize(math.erf)
    h = 0.5 * h * (1.0 + erf(h / math.sqrt(2.0)))
    return h @ W2e.astype(np.float64) + b2e.astype(np.float64)


def _have_axon_devices():
    try:
        import jax

        return (
            sum(d.platform in ("axon", "neuron") for d in jax.devices()) >= _NCORES
        )
    except Exception:
        return False


def _prepare(inputs):
    import ml_dtypes

    bf16 = ml_dtypes.bfloat16
    x = np.asarray(inputs["x"], np.float32)
    Wr = np.asarray(inputs["Wr"], np.float32)
    br = np.asarray(inputs["br"], np.float32)
    W1 = np.asarray(inputs["W1"], np.float32)
    b1 = np.asarray(inputs["b1"], np.float32)
    W2 = np.asarray(inputs["W2"], np.float32)
    b2 = np.asarray(inputs["b2"], np.float32)
    k = int(np.asarray(inputs["k"]))
    assert x.shape == (_B, _SEQ, _E), x.shape

    flat = x.reshape(_T, _E)
    topk = _route(flat, Wr, br, k)
    flatT = np.ascontiguousarray(flat.T)

    in_maps = []
    idx_list = []
    overflow = []
    for e in range(_NE):
        idx = np.nonzero((topk == e).any(axis=1))[0]
        if len(idx) > _CAP:
            overflow.append((e, idx[_CAP:]))
            idx = idx[:_CAP]
        idx_list.append(idx)
        xt = np.zeros((_E, _CAP), bf16)
        xt[:, : len(idx)] = flatT[:, idx]
        in_maps.append(
            {
                "xt": xt,
                "w1": W1[e].astype(bf16),
                "w2k": (W2[e] / k).astype(bf16),
                "b1p": np.ascontiguousarray(b1[e].reshape(_H // _P, _P).T),
                "b2r": np.broadcast_to(b2[e] / k, (_P, _E)).copy(),
            }
        )
    return flat, k, in_maps, idx_list, overflow, (W1, b1, W2, b2)


def kernel(**inputs) -> np.ndarray:
    flat, k, in_maps, idx_list, overflow, wb = _prepare(inputs)
    if not _have_axon_devices():
        # no trn2 cores visible — compute on host so we still return the
        # right answer
        W1, b1, W2, b2 = wb
        out = np.zeros((_T, _E), np.float64)
        for e in range(_NE):
            idx = idx_list[e]
            out[idx] += _host_expert(flat[idx], W1[e], b1[e], W2[e], b2[e]) / k
        for e, idx in overflow:
            out[idx] += _host_expert(flat[idx], W1[e], b1[e], W2[e], b2[e]) / k
        return out.astype(np.float32).reshape(_B, _SEQ, _E)
    if overflow:
        # recompute overflow rows fully on host (exact erf gelu)
        W1, b1, W2, b2 = wb
        extra = [(e, idx, _host_expert(flat[idx], W1[e], b1[e], W2[e], b2[e]) / k)
                 for e, idx in overflow]
    else:
        extra = []

    key = (float(1.0 / k),)
    if key not in _nc_cache:
        nc = _build_nc(1.0 / k)
        _nc_cache[key] = _make_runner(nc, _NCORES)
    run = _nc_cache[key]
    results = run(in_maps)

    out = np.zeros((_T, _E), np.float32)
    for e in range(_NE):
        y = results[e]["y"]
        n = len(idx_list[e])
        out[idx_list[e]] += y[:n].astype(np.float32)
    for e, idx, yv in extra:
        out[idx] += yv.astype(np.float32)
    return out.reshape(_B, _SEQ, _E)


# revision 9
# speedup vs baseline: 1.5112x; 1.0042x over previous
"""MoE (top-2 of 8 experts) Trainium2 kernel.

Strategy: expert-parallel across 8 NeuronCores with token rebalancing.
The router (8192x1024 @ 1024x8 + top-k) is tiny, so it runs on host in
float64 (verified to reproduce the fp32 reference ranking). Each core
processes 2176 token slots = 16 "primary" blocks of 128 for its own
expert plus 1 "guest" block carrying another expert's overflow (the
per-expert token counts for the benchmark routing are 1868..2252, so a
uniform 18-block capacity wastes ~6% PE time; 17 blocks with guest
rebalancing is the minimum uniform SPMD capacity). The FFN runs in bf16
(full PE rate, fast weight load, half the DMA bytes of fp32; ~4e-3 rel
err vs the 2e-2 gate) with exact-erf Gelu on ScalarE; the host
scatter-adds the two expert contributions per token.

Device layout: stage 1 computes h.T = gelu(W1.T @ x.T + b1) with W1
blocks stationary; stage 2 uses h.T 128x128 blocks as the stationary
operand streaming two 512-wide W2 chunks per load (halving weight-load
count) and produces y directly in [token, E] layout. The 1/k scale is
folded into W2 on host (exact for k=2); b2/k is added with a DVE
tensor_add from a partition-replicated tile.

DMA queues: token/weight loads go out on the sync engine's HWDGE ring;
y stores ride the otherwise-idle gpsimd (SWDGE) queue, so input
prefetch is never queued behind store triggers that wait on compute.
The guest expert's weights are DMA'd behind the primary set and only
consumed at the end of the pass, hiding their load under primary
compute. PSUM: 4 banks for stage-1 accumulation chains, 4 for stage-2
so consecutive 128-token blocks alternate bank pairs and the PE never
waits on the DVE drain.
"""

import sys

sys.path.insert(0, "/opt/trn_rl_repo")

import math

import numpy as np

_B, _SEQ, _E, _H, _NE = 4, 2048, 1024, 1536, 8
_T = _B * _SEQ
_CAPP = 2048  # primary region (16 blocks of 128, own expert)
_CAPG = 128   # guest region (1 block, possibly another expert's overflow)
_CAP = _CAPP + _CAPG
_NCORES = 8
_P = 128

_nc_cache: dict = {}
_LOOP_BODY_REPS = [1]  # timing-only knob for the For_i variant


def _build_nc(
    inv_k: float,
    repeat: int = 1,
    loop_n: int = 0,
    ps1_bufs: int = 4,
    ps2_bufs: int = 4,
):
    """repeat>1 repeats the compute (timing); repeat=0 builds an I/O-identical
    near-no-op NEFF used as the timing baseline; loop_n>0 wraps the pass in a
    device-side For_i loop (timing only)."""
    from contextlib import ExitStack

    import concourse.tile as tile
    from concourse import bacc, mybir

    f32 = mybir.dt.float32
    bf = mybir.dt.bfloat16
    KO1 = _E // _P   # 8  k-tiles for layer-1 contraction
    HT = _H // _P    # 12 h-tiles (layer-1 out / layer-2 contraction)
    EC = _E // 512   # 2  512-wide E chunks in stage 2
    colt = [(0, 512), (512, 512), (1024, 512), (1536, 512)]  # primary tiles
    gcol = (_CAPP, _CAPG)                                    # guest tile

    nc = bacc.Bacc("TRN2", target_bir_lowering=False, debug=False)
    xt_d = nc.dram_tensor("xt", [_E, _CAP], bf, kind="ExternalInput").ap()
    # a = primary expert weights, b = guest expert weights
    # w2 is host-scaled by 1/k (exact for k a power of two); b1 is laid out
    # [128, HT] (bias hi per column), b2 replicated across partitions.
    wd = {}
    for s in ("a", "b"):
        wd[s] = dict(
            w1=nc.dram_tensor(f"w1{s}", [_E, _H], bf, kind="ExternalInput").ap(),
            w2=nc.dram_tensor(f"w2{s}", [_H, _E], bf, kind="ExternalInput").ap(),
            b1=nc.dram_tensor(f"b1{s}", [_P, HT], f32, kind="ExternalInput").ap(),
            b2=nc.dram_tensor(f"b2{s}", [_P, _E], f32, kind="ExternalInput").ap(),
        )
    # y in natural [token, E] layout
    y_d = nc.dram_tensor("y", [_CAP, _E], bf, kind="ExternalOutput").ap()

    with tile.TileContext(nc) as tc:
        with ExitStack() as ctx:
            wpool = ctx.enter_context(tc.tile_pool(name="w", bufs=1))
            cpool = ctx.enter_context(tc.tile_pool(name="c", bufs=1))
            xpool = ctx.enter_context(tc.tile_pool(name="x", bufs=5))
            hpool = ctx.enter_context(tc.tile_pool(name="h", bufs=2))
            ypool = ctx.enter_context(tc.tile_pool(name="y", bufs=4))
            ps1 = ctx.enter_context(tc.tile_pool(name="ps1", bufs=ps1_bufs, space="PSUM"))
            ps2 = ctx.enter_context(tc.tile_pool(name="ps2", bufs=ps2_bufs, space="PSUM"))

            if repeat == 0 and not loop_n:
                t = cpool.tile([_P, 4], f32, tag="nop")
                nc.sync.dma_start(t[:], wd["a"]["b1"][:, 0:4])
                o = cpool.tile([_P, 4], f32, tag="nop_o")
                nc.vector.tensor_copy(o[:], t[:])
                nc.gpsimd.dma_start(y_d[0:_P, 0:4], o[:])
            else:
                ws = {}
                for s in ("a", "b"):
                    ws[s] = dict(
                        w1=[wpool.tile([_P, _H], bf, tag=f"w1{s}_{ko}",
                                       name=f"w1{s}_{ko}") for ko in range(KO1)],
                        w2=[wpool.tile([_P, _E], bf, tag=f"w2{s}_{hi}",
                                       name=f"w2{s}_{hi}") for hi in range(HT)],
                        b1=cpool.tile([_P, HT], f32, tag=f"b1{s}", name=f"b1{s}_sb"),
                        b2=cpool.tile([_P, _E], f32, tag=f"b2{s}", name=f"b2{s}_sb"),
                    )

                def dma_w(s, parts=("w1", "b1", "w2", "b2")):
                    for p in parts:
                        if p == "w1":
                            for ko in range(KO1):
                                nc.sync.dma_start(
                                    ws[s]["w1"][ko][:],
                                    wd[s]["w1"][ko * _P : (ko + 1) * _P, :],
                                )
                        elif p == "w2":
                            for hi in range(HT):
                                nc.sync.dma_start(
                                    ws[s]["w2"][hi][:],
                                    wd[s]["w2"][hi * _P : (hi + 1) * _P, :],
                                )
                        else:
                            nc.sync.dma_start(ws[s][p][:], wd[s][p][:, :])

                def dma_xt(c0, nt, tag2, interleave=None, ilv_first=False):
                    tiles = []
                    for ko in range(KO1):
                        if interleave is not None and ilv_first:
                            interleave(ko)
                        tt = xpool.tile([_P, nt], bf, tag=f"xt{ko}_{nt}",
                                        name=f"xt_{tag2}_{ko}")
                        nc.sync.dma_start(
                            tt[:], xt_d[ko * _P : (ko + 1) * _P, c0 : c0 + nt]
                        )
                        if interleave is not None and not ilv_first:
                            interleave(ko)
                        tiles.append(tt)
                    return lambda ko: tiles[ko][:]

                def alloc_h(nt, tag2):
                    tiles = [
                        hpool.tile([_P, nt], bf, tag=f"h{hi}_{nt}",
                                   name=f"h_{tag2}_{hi}")
                        for hi in range(HT)
                    ]
                    return (lambda hi: tiles[hi][:],
                            lambda hk, cb: tiles[hk][:, cb * _P : (cb + 1) * _P])

                def stage1_koouter(xt_sb, nt, s):
                    # ko-outer half-passes: PE starts as soon as w1 block 0 lands
                    h_w, h_r = alloc_h(nt, "t0")
                    for half in range(2):
                        accs = [
                            ps1.tile([_P, nt], f32, tag="ps1", name=f"ps_h{half}_{i}")
                            for i in range(6)
                        ]
                        for ko in range(KO1):
                            for i in range(6):
                                hi = half * 6 + i
                                nc.tensor.matmul(
                                    accs[i][:],
                                    ws[s]["w1"][ko][:, hi * _P : (hi + 1) * _P],
                                    xt_sb(ko),
                                    start=(ko == 0),
                                    stop=(ko == KO1 - 1),
                                )
                        for i in range(6):
                            hi = half * 6 + i
                            nc.scalar.activation(
                                h_w(hi),
                                accs[i][:],
                                mybir.ActivationFunctionType.Gelu,
                                bias=ws[s]["b1"][:, hi : hi + 1],
                                scale=1.0,
                            )
                    return h_w, h_r

                def stage1(xt_sb, nt, tag2, s):
                    h_w, h_r = alloc_h(nt, tag2)
                    for hi in range(HT):
                        acc = ps1.tile([_P, nt], f32, tag="ps1", name=f"p1_{tag2}_{hi}")
                        for ko in range(KO1):
                            nc.tensor.matmul(
                                acc[:],
                                ws[s]["w1"][ko][:, hi * _P : (hi + 1) * _P],
                                xt_sb(ko),
                                start=(ko == 0),
                                stop=(ko == KO1 - 1),
                            )
                        nc.scalar.activation(
                            h_w(hi),
                            acc[:],
                            mybir.ActivationFunctionType.Gelu,
                            bias=ws[s]["b1"][:, hi : hi + 1],
                            scale=1.0,
                        )
                    return h_w, h_r

                def stage2(h_r, c0, nt, tag2, s):
                    # stationary = h.T 128x128 block (one load, 2 MMs of N=512)
                    for cb in range(nt // _P):
                        accs = [
                            ps2.tile([_P, 512], f32, tag="ps2",
                                     name=f"p2_{tag2}_{cb}_{ec}")
                            for ec in range(EC)
                        ]
                        for hk in range(HT):
                            lhs = h_r(hk, cb)
                            for ec in range(EC):
                                nc.tensor.matmul(
                                    accs[ec][:],
                                    lhs,
                                    ws[s]["w2"][hk][:, ec * 512 : (ec + 1) * 512],
                                    start=(hk == 0),
                                    stop=(hk == HT - 1),
                                )
                        for ec in range(EC):
                            y_sb = ypool.tile([_P, 512], bf, tag="y",
                                              name=f"y_{tag2}_{cb}_{ec}")
                            nc.vector.tensor_add(
                                y_sb[:],
                                accs[ec][:],
                                ws[s]["b2"][:, ec * 512 : (ec + 1) * 512],
                            )
                            nc.gpsimd.dma_start(
                                y_d[
                                    c0 + cb * _P : c0 + (cb + 1) * _P,
                                    ec * 512 : (ec + 1) * 512,
                                ],
                                y_sb[:],
                            )

                def emit_pass(tag):
                    xts = [dma_xt(c0, nt, f"{tag}_{c0}") for c0, nt in colt]
                    xtg = dma_xt(gcol[0], gcol[1], f"{tag}_g")
                    for t, (c0, nt) in enumerate(colt):
                        _, h_r = stage1(xts[t], nt, f"{tag}_{c0}", "a")
                        stage2(h_r, c0, nt, f"{tag}_{c0}", "a")
                    _, h_r = stage1(xtg, gcol[1], f"{tag}_g", "b")
                    stage2(h_r, gcol[0], gcol[1], f"{tag}_g", "b")

                if loop_n:
                    # weights resident; body re-loads xt and writes y
                    dma_w("a")
                    dma_w("b")
                    body_reps = int(_LOOP_BODY_REPS[0])
                    with tc.For_i(0, loop_n, 1) as _i:
                        for br in range(body_reps):
                            emit_pass(f"L{br}")
                else:
                    for r in range(repeat):
                        if r == 0:
                            # prelude issue order: w1a/xt0 interleaved so the PE
                            # can start immediately; b1a; then xt1..3 with w2a
                            # trickled in between; b2a; guest xt; the guest
                            # weight set last (consumed at the end of the pass).
                            xt0 = dma_xt(
                                colt[0][0], colt[0][1], "r0t0",
                                interleave=lambda ko: nc.sync.dma_start(
                                    ws["a"]["w1"][ko][:],
                                    wd["a"]["w1"][ko * _P : (ko + 1) * _P, :],
                                ),
                                ilv_first=True,
                            )
                            nc.sync.dma_start(ws["a"]["b1"][:], wd["a"]["b1"][:, :])
                            xts = [xt0]
                            w2i = iter(range(HT))

                            def ilv(ko):
                                hi = next(w2i, None)
                                if hi is not None:
                                    nc.sync.dma_start(
                                        ws["a"]["w2"][hi][:],
                                        wd["a"]["w2"][hi * _P : (hi + 1) * _P, :],
                                    )

                            for c0, nt in colt[1:]:
                                xts.append(dma_xt(c0, nt, f"r0c{c0}", interleave=ilv))
                            for hi in w2i:
                                nc.sync.dma_start(
                                    ws["a"]["w2"][hi][:],
                                    wd["a"]["w2"][hi * _P : (hi + 1) * _P, :],
                                )
                            nc.sync.dma_start(ws["a"]["b2"][:], wd["a"]["b2"][:, :])
                            xtg = dma_xt(gcol[0], gcol[1], "r0g")
                            dma_w("b")
                            _, h0r = stage1_koouter(xts[0], colt[0][1], "a")
                            _, h1r = stage1(xts[1], colt[1][1], "r0t1", "a")
                            stage2(h0r, colt[0][0], colt[0][1], "r0t0", "a")
                            stage2(h1r, colt[1][0], colt[1][1], "r0t1", "a")
                            for t, (c0, nt) in enumerate(colt[2:], start=2):
                                _, h_r = stage1(xts[t], nt, f"r0c{c0}", "a")
                                stage2(h_r, c0, nt, f"r0c{c0}", "a")
                            _, h_r = stage1(xtg, gcol[1], "r0g", "b")
                            stage2(h_r, gcol[0], gcol[1], "r0g", "b")
                        else:
                            emit_pass(f"r{r}")
    nc.compile()
    return nc


def _make_runner(nc, n_cores):
    """Persistent-jit SPMD runner (modeled on bass2jax.run_bass_via_pjrt)."""
    import jax
    import numpy as _np
    from jax.sharding import Mesh, PartitionSpec
    from jax.experimental.shard_map import shard_map

    from concourse import mybir
    from concourse.bass2jax import (
        _bass_exec_p,
        install_neuronx_cc_hook,
        partition_id_tensor,
    )

    install_neuronx_cc_hook()

    partition_name = nc.partition_id_tensor.name if nc.partition_id_tensor else None
    in_names: list = []
    out_names: list = []
    out_avals: list = []
    zero_outs: list = []
    for alloc in nc.m.functions[0].allocations:
        if not isinstance(alloc, mybir.MemoryLocationSet):
            continue
        name = alloc.memorylocations[0].name
        if alloc.kind == "ExternalInput":
            if name != partition_name:
                in_names.append(name)
        elif alloc.kind == "ExternalOutput":
            shape = tuple(alloc.tensor_shape)
            dtype = mybir.dt.np(alloc.dtype)
            out_names.append(name)
            out_avals.append(jax.core.ShapedArray(shape, dtype))
            zero_outs.append(_np.zeros(shape, dtype))
    n_params = len(in_names)
    n_outs = len(out_avals)
    all_in_names = in_names + out_names
    if partition_name is not None:
        all_in_names = all_in_names + [partition_name]

    def _body(*args):
        operands = list(args)
        if partition_name is not None:
            operands.append(partition_id_tensor())
        outs = _bass_exec_p.bind(
            *operands,
            out_avals=tuple(out_avals),
            in_names=tuple(all_in_names),
            out_names=tuple(out_names),
            lowering_input_output_aliases=(),
            sim_require_finite=True,
            sim_require_nnan=True,
            nc=nc,
        )
        return tuple(outs)

    devices = jax.devices()[:n_cores]
    assert len(devices) == n_cores
    mesh = Mesh(_np.asarray(devices), ("core",))
    in_specs = (PartitionSpec("core"),) * (n_params + n_outs)
    out_specs = (PartitionSpec("core"),) * n_outs
    donate = tuple(range(n_params, n_params + n_outs))
    sharded = jax.jit(
        shard_map(
            _body, mesh=mesh, in_specs=in_specs, out_specs=out_specs, check_rep=False
        ),
        donate_argnums=donate,
        keep_unused=True,
    )

    def run(in_maps):
        concat_in = [
            _np.concatenate([_np.asarray(in_maps[c][nm]) for c in range(n_cores)], axis=0)
            for nm in in_names
        ]
        concat_zeros = [
            _np.zeros((n_cores * z.shape[0], *z.shape[1:]), z.dtype) for z in zero_outs
        ]
        out_arrs = sharded(*concat_in, *concat_zeros)
        out_arrs = [_np.asarray(o) for o in out_arrs]
        return [
            {
                nm: out_arrs[i].reshape(n_cores, *out_avals[i].shape)[c]
                for i, nm in enumerate(out_names)
            }
            for c in range(n_cores)
        ]

    return run


def _route(flat, Wr, br, k):
    logits = flat.astype(np.float64) @ Wr.astype(np.float64) + br.astype(np.float64)
    order = np.argsort(-logits, axis=1, kind="stable")
    return order[:, :k]


def _host_expert(xe, W1e, b1e, W2e, b2e):
    h = xe.astype(np.float64) @ W1e.astype(np.float64) + b1e.astype(np.float64)
    try:
        from scipy.special import erf
    except ImportError:
        erf = np.vectorize(math.erf)
    h = 0.5 * h * (1.0 + erf(h / math.sqrt(2.0)))
    return h @ W2e.astype(np.float64) + b2e.astype(np.float64)


def _have_axon_devices():
    try:
        import jax

        return (
            sum(d.platform in ("axon", "neuron") for d in jax.devices()) >= _NCORES
        )
    except Exception:
        return False


def _pack(idx_lists):
    """Assign tokens to cores: expert c's first 2048 tokens are core c's
    primary region; overflow is split into <=128-token chunks, one per
    guest slot, round-robin over cores. Returns per-core primary/guest
    index arrays plus any chunks that didn't fit (host fallback)."""
    prim = []
    chunks = []
    for e in range(_NE):
        idx = idx_lists[e]
        prim.append(idx[:_CAPP])
        rest = idx[_CAPP:]
        for i in range(0, len(rest), _CAPG):
            chunks.append((e, rest[i : i + _CAPG]))
    guest = [None] * _NCORES
    unplaced = []
    free = list(range(_NCORES))
    for ch in chunks:
        if free:
            guest[free.pop(0)] = ch
        else:
            unplaced.append(ch)
    return prim, guest, unplaced


def _prepare(inputs):
    import ml_dtypes

    bf16 = ml_dtypes.bfloat16
    x = np.asarray(inputs["x"], np.float32)
    Wr = np.asarray(inputs["Wr"], np.float32)
    br = np.asarray(inputs["br"], np.float32)
    W1 = np.asarray(inputs["W1"], np.float32)
    b1 = np.asarray(inputs["b1"], np.float32)
    W2 = np.asarray(inputs["W2"], np.float32)
    b2 = np.asarray(inputs["b2"], np.float32)
    k = int(np.asarray(inputs["k"]))
    assert x.shape == (_B, _SEQ, _E), x.shape

    flat = x.reshape(_T, _E)
    topk = _route(flat, Wr, br, k)
    flatT = np.ascontiguousarray(flat.T)

    idx_lists = [np.nonzero((topk == e).any(axis=1))[0] for e in range(_NE)]
    prim, guest, unplaced = _pack(idx_lists)

    w1b = [W1[e].astype(bf16) for e in range(_NE)]
    w2b = [(W2[e] / k).astype(bf16) for e in range(_NE)]
    b1p = [np.ascontiguousarray(b1[e].reshape(_H // _P, _P).T) for e in range(_NE)]
    b2r = [np.broadcast_to(b2[e] / k, (_P, _E)).copy() for e in range(_NE)]

    in_maps = []
    for c in range(_NE):
        xt = np.zeros((_E, _CAP), bf16)
        xt[:, : len(prim[c])] = flatT[:, prim[c]]
        ge = c
        if guest[c] is not None:
            ge, gidx = guest[c]
            xt[:, _CAPP : _CAPP + len(gidx)] = flatT[:, gidx]
        in_maps.append(
            {
                "xt": xt,
                "w1a": w1b[c], "w2a": w2b[c], "b1a": b1p[c], "b2a": b2r[c],
                "w1b": w1b[ge], "w2b": w2b[ge], "b1b": b1p[ge], "b2b": b2r[ge],
            }
        )
    return flat, k, in_maps, prim, guest, unplaced, (W1, b1, W2, b2)


def kernel(**inputs) -> np.ndarray:
    flat, k, in_maps, prim, guest, unplaced, wb = _prepare(inputs)
    W1, b1, W2, b2 = wb
    if not _have_axon_devices():
        # no trn2 cores visible — compute on host so we still return the
        # right answer
        out = np.zeros((_T, _E), np.float64)
        for c in range(_NCORES):
            out[prim[c]] += _host_expert(flat[prim[c]], W1[c], b1[c], W2[c], b2[c]) / k
            if guest[c] is not None:
                ge, gidx = guest[c]
                out[gidx] += _host_expert(flat[gidx], W1[ge], b1[ge], W2[ge], b2[ge]) / k
        for e, idx in unplaced:
            out[idx] += _host_expert(flat[idx], W1[e], b1[e], W2[e], b2[e]) / k
        return out.astype(np.float32).reshape(_B, _SEQ, _E)
    # tokens that didn't fit the device layout (none for the benchmark
    # routing) are recomputed exactly on host
    extra = [(idx, _host_expert(flat[idx], W1[e], b1[e], W2[e], b2[e]) / k)
             for e, idx in unplaced]

    key = (float(1.0 / k),)
    if key not in _nc_cache:
        nc = _build_nc(1.0 / k)
        _nc_cache[key] = _make_runner(nc, _NCORES)
    run = _nc_cache[key]
    results = run(in_maps)

    out = np.zeros((_T, _E), np.float32)
    for c in range(_NCORES):
        y = results[c]["y"]
        out[prim[c]] += y[: len(prim[c])].astype(np.float32)
        if guest[c] is not None:
            ge, gidx = guest[c]
            out[gidx] += y[_CAPP : _CAPP + len(gidx)].astype(np.float32)
    for idx, yv in extra:
        out[idx] += yv.astype(np.float32)
    return out.reshape(_B, _SEQ, _E)
